# revision 28
# baseline (speedup 1.0000x reference)
"""Causal self-attention (B=2, T=4096, C=768, NH=12) on 8 trn2 NeuronCores.

Sharding: data parallel on B (cores 0-3 -> batch 0, cores 4-7 -> batch 1),
tensor parallel on heads (3 heads per core).  Each core computes, for its
batch b and heads [h0, h0+1, h0+2]:
  qkv^T = (x_b @ W_slice)^T           (f32r matmuls, W stationary)
  per head: causal attention entirely in SBUF
      s^T[k,q] = k^T . q^T   (scores transposed; softmax denom via a ones
      column appended to v so one PV matmul yields both y^T and denom)
  y^T normalized via Newton-iteration reciprocal + PE broadcast
  z^T_partial = (y @ Wp_rows)^T       (per-core partial of output proj)
Host sums the 4 partials per batch and adds the biases' contribution.

The whole kernel is one software pipeline over the 8 token chunks:
qkv-projection(chunk) -> v-transposes(chunk) -> attention(q-chunk) ->
normalize -> z-projection(chunk), so PE / ACT / DVE / DMA overlap
throughout instead of phase-by-phase.

All matmuls use dt.float32r (tf32: 1 cyc/row at N>=256, fp32 accumulate).
"""

import sys

for _p in ("/opt/trn_rl_repo",):
    if _p not in sys.path:
        sys.path.insert(0, _p)

import numpy as np
from contextlib import ExitStack

from concourse import bacc, tile, mybir

B, T, C = 2, 4096, 768
NH, HD = 12, 64
NCORES = 8
HPC = 3            # heads per core
QC = 512           # q chunk width (N dim of matmuls)
NQC = T // QC      # 8
KB = 128           # k block (partition dim of transposed scores)
NKB = T // KB      # 32
KW = 3 * HPC * HD  # 576 qkv columns per core
VS = HD + 4        # vones block stride (8B-aligned for bf16)
f32 = mybir.dt.float32
f32r = mybir.dt.float32r
bf16 = mybir.dt.bfloat16
u32 = mybir.dt.uint32
u16 = mybir.dt.uint16

MAGIC = 0x7EF311C3  # fp32 reciprocal seed: r0 = bits(MAGIC - bits(x))


def build_program():
    nc = bacc.Bacc("TRN2", target_bir_lowering=False, debug=False,
                   num_devices=NCORES)
    xT = nc.dram_tensor("xT", [128, 6, T], bf16, kind="ExternalInput").ap()
    wqkv = nc.dram_tensor("wqkv", [128, 6 * KW], bf16,
                          kind="ExternalInput").ap()
    bqkv = nc.dram_tensor("bqkv", [128, 5], f32, kind="ExternalInput").ap()
    wp = nc.dram_tensor("wp", [HPC * HD, C], bf16, kind="ExternalInput").ap()
    masks = nc.dram_tensor("masks", [KB, 4 * QC], bf16,
                           kind="ExternalInput").ap()
    eye = nc.dram_tensor("eye", [2 * HD, HD], bf16, kind="ExternalInput").ap()
    zT = nc.dram_tensor("zT", [128, 6, T], bf16,
                        kind="ExternalOutput").ap()

    with tile.TileContext(nc) as tc:
        _body(tc, xT, wqkv, bqkv, wp, masks, eye, zT)
    nc.compile()
    return nc


def _body(tc, xT, wqkv, bqkv, wp, masks, eye, zT):
    nc = tc.nc
    Exp = mybir.ActivationFunctionType.Exp
    mult = mybir.AluOpType.mult
    sub = mybir.AluOpType.subtract
    add = mybir.AluOpType.add

    with ExitStack() as ctx:
        # ---------------- SBUF pools (all live for the whole pipeline) ----
        qk_pool = ctx.enter_context(tc.tile_pool(name="qk", bufs=4))
        y_pool = ctx.enter_context(tc.tile_pool(name="yt", bufs=2))
        wp_pool = ctx.enter_context(tc.tile_pool(name="wp", bufs=2))
        wq_pool = ctx.enter_context(tc.tile_pool(name="wq", bufs=1))
        mask_pool = ctx.enter_context(tc.tile_pool(name="mask", bufs=1))
        const_pool = ctx.enter_context(tc.tile_pool(name="const", bufs=1))
        bias_pool = ctx.enter_context(tc.tile_pool(name="bias", bufs=1))
        vones_pool = ctx.enter_context(tc.tile_pool(name="vones", bufs=1))
        xk_pool = ctx.enter_context(tc.tile_pool(name="xk", bufs=2))
        vt_pool = ctx.enter_context(tc.tile_pool(name="vt", bufs=2))
        p_pool = ctx.enter_context(tc.tile_pool(name="p", bufs=4))
        z_pool = ctx.enter_context(tc.tile_pool(name="z", bufs=2))
        d_pool = ctx.enter_context(tc.tile_pool(name="d", bufs=1))
        # PSUM pools: 4 + 2 + 2 = 8 banks
        s_ps = ctx.enter_context(tc.tile_pool(name="sps", bufs=2, space="PSUM"))
        y_ps = ctx.enter_context(tc.tile_pool(name="yps", bufs=2, space="PSUM"))
        mm_ps = ctx.enter_context(tc.tile_pool(name="mmps", bufs=2,
                                               space="PSUM"))

        # persistent tiles
        # q^T/k^T layout: A=[q_l0;q_l1] B=[k_l0;k_l1] Cqk=[q_l2;k_l2]
        # E2 = copy of k_l2 at base partition 0 (q2 and k2 must share a base)
        A = qk_pool.tile([128, T], bf16, tag="qk", name="qk")
        Bt = qk_pool.tile([128, T], bf16, tag="qk", name="qk")
        Cqk = qk_pool.tile([128, T], bf16, tag="qk", name="qk")
        E2 = qk_pool.tile([128, T], bf16, tag="qk", name="qk")
        F = y_pool.tile([128, T], bf16, tag="yt", name="yt")   # y^T l0,l1
        G = y_pool.tile([64, T], bf16, tag="yt", name="yt")    # y^T l2
        wp0 = wp_pool.tile([128, C], bf16, tag="wp", name="wp")
        wp1 = wp_pool.tile([64, C], bf16, tag="wp", name="wp")
        mask_t = mask_pool.tile([KB, 4 * QC], bf16, tag="mask", name="mask")
        eye_t = const_pool.tile([2 * HD, HD], bf16, tag="eye", name="eye")
        ones_pe = const_pool.tile([1, 64], bf16, tag="ones", name="ones")
        vones = [vones_pool.tile([128, NKB * VS], bf16, tag=f"vones{i}",
                                 name=f"vones{i}") for i in range(HPC)]

        wq_all = wq_pool.tile([128, 6 * KW], bf16, tag="wq", name="wq")
        nc.sync.dma_start(out=wq_all[:], in_=wqkv[:])
        wq_t = [wq_all[:, kt * KW:(kt + 1) * KW] for kt in range(6)]
        bias_all = bias_pool.tile([128, 5], f32, tag="bias", name="bias")
        nc.sync.dma_start(out=bias_all[:], in_=bqkv[:])
        bias_t = [bias_all[0:(128 if m < 4 else 64), m:m + 1]
                  for m in range(5)]
        nc.sync.dma_start(out=eye_t[:], in_=eye[:])
        nc.gpsimd.memset(ones_pe[:].bitcast(u16), 0x3F80)  # +1.0bf
        for h in range(HPC):
            nc.gpsimd.memset(vones[h][:].bitcast(u16), 0x3F80)  # 1.0bf

        qmap = [A[0:64, :], A[64:128, :], Cqk[0:64, :]]
        kmap = [Bt[0:64, :], Bt[64:128, :], E2[0:64, :]]

        def finish_chunk(item):
            qc, dsl, rrows = item
            for h in range(HPC):
                rb = mm_ps.tile([128, QC], f32, tag="mm", name="mm")
                nc.tensor.matmul(rb[0:64, :], lhsT=ones_pe[:],
                                 rhs=rrows[h][:], start=True, stop=True)
                if h == 0:
                    ydst = F[0:64, dsl]
                elif h == 1:
                    ydst = F[64:128, dsl]
                else:
                    ydst = G[0:64, dsl]
                nc.vector.tensor_tensor(out=ydst, in0=ydst, in1=rb[0:64, :],
                                        op=mult)
            zbig = z_pool.tile([128, 6 * QC], bf16, tag="z", name="z")
            Copy = mybir.ActivationFunctionType.Copy
            for mc in range(6):
                zp = mm_ps.tile([128, QC], f32, tag="mm", name="mm")
                nc.tensor.matmul(zp[:],
                                 lhsT=wp0[:, mc * 128:(mc + 1) * 128],
                                 rhs=F[:, dsl], start=True, stop=False)
                nc.tensor.matmul(zp[:],
                                 lhsT=wp1[:, mc * 128:(mc + 1) * 128],
                                 rhs=G[:, dsl], start=False, stop=True)
                dst = zbig[:, mc * QC:(mc + 1) * QC]
                if mc % 2 == 0:   # split PSUM drains across DVE and ACT
                    nc.vector.tensor_copy(out=dst, in_=zp[:])
                else:
                    nc.scalar.activation(dst, zp[:], Copy)
            nc.sync.dma_start(out=zT[:, :, dsl], in_=zbig[:])

        # ================= the pipeline over token chunks =================
        pending = []
        for tci in range(NQC):
            csl = slice(tci * QC, (tci + 1) * QC)

            # ---- qkv^T projection for this chunk (one packed DMA) ----
            xk_all = xk_pool.tile([128, 6 * QC], bf16, tag="xk", name="xk")
            nc.sync.dma_start(out=xk_all[:], in_=xT[:, :, csl])
            if tci == 0:
                # off the lead-in critical path: masks are first consumed by
                # attention(0) (after qkv proj), wp only by finish_chunk(0)
                # during iteration 1.
                nc.sync.dma_start(out=mask_t[:], in_=masks[:])
                nc.sync.dma_start(out=wp0[:], in_=wp[0:128, :])
                nc.sync.dma_start(out=wp1[:], in_=wp[128:192, :])
            vchunk = vt_pool.tile([128, QC], bf16, tag="vt", name="vt")
            vchunk2 = vt_pool.tile([64, QC], bf16, tag="vt2", name="vt2")
            mdst = [(A, 0, 128), (Cqk, 0, 128), (Bt, 0, 128),
                    (vchunk, 0, 128), (vchunk2, 0, 64)]
            for m in range(5):
                dst, roff, pm = mdst[m]
                ps = mm_ps.tile([128, QC], f32, tag="mm", name="mm")
                for kt in range(6):
                    nc.tensor.matmul(
                        ps[0:pm, :],
                        lhsT=wq_t[kt][:, m * 128:m * 128 + pm],
                        rhs=xk_all[:, kt * QC:(kt + 1) * QC],
                        start=(kt == 0), stop=(kt == 5))
                out_sl = dst[roff:roff + pm, csl] if m < 3 else dst[0:pm, :]
                nc.vector.tensor_scalar(out_sl, ps[0:pm, :], bias_t[m][:],
                                        None, add)
                if m == 1:   # k_l2 at base 0 and q_l2 at base 64 (pairing)
                    nc.vector.tensor_scalar(
                        E2[0:64, csl], ps[64:128, :],
                        bias_t[1][64:128, :], None, add)
                    nc.vector.tensor_scalar(
                        E2[64:128, csl], ps[0:64, :],
                        bias_t[1][0:64, :], None, add)

            # ---- v transposes for this chunk's 4 k-blocks ----
            vmap = [vchunk[0:64, :], vchunk[64:128, :], vchunk2[0:64, :]]
            for jj in range(4):
                j = 4 * tci + jj
                tps = []
                for h in range(2):   # h0/h1 adjacent -> row-concurrent
                    base = vmap[h].base_partition()
                    tp = mm_ps.tile([128, HD], bf16, tag="mm", name="mm")
                    nc.tensor.transpose(
                        tp[:], vmap[h][:, jj * KB:(jj + 1) * KB],
                        eye_t[base:base + HD, :])
                    tps.append(tp)
                for h in range(2):
                    nc.vector.tensor_copy(
                        out=vones[h][:, j * VS:j * VS + HD], in_=tps[h][:])
                tp2 = mm_ps.tile([128, HD], bf16, tag="mm", name="mm")
                nc.tensor.transpose(tp2[:], vmap[2][:, jj * KB:(jj + 1) * KB],
                                    eye_t[0:HD, :])
                nc.vector.tensor_copy(
                    out=vones[2][:, j * VS:j * VS + HD], in_=tp2[:])

            # ---- attention for q chunk qc == tci ----
            qc = tci
            dsl = csl
            nkb = 4 * qc + 4
            rrows = []

            def apply_masks(pt, js):
                for si, j in enumerate(js):
                    m = j - 4 * qc
                    if m >= 0:   # diagonal block: apply causal mask
                        nc.vector.tensor_tensor(
                            out=pt[:, si * QC:(si + 1) * QC],
                            in0=pt[:, si * QC:(si + 1) * QC],
                            in1=mask_t[:, m * QC:(m + 1) * QC],
                            op=mult)

            def drain_y(h, yps):
                ydst = (F[0:64, dsl], F[64:128, dsl], G[0:64, dsl])[h]
                nc.vector.tensor_copy(out=ydst, in_=yps[0:64, :])
                # reciprocal straight off the PSUM denominator row into a
                # partition-0 tile: replaces the whole Newton chain and the
                # partition-gather DMAs.
                rr = d_pool.tile([1, QC], f32, tag=f"rcp{h}",
                                 name=f"rcp{h}", bufs=2)
                nc.vector.reciprocal(out=rr[:], in_=yps[64:65, :])
                rrb = d_pool.tile([1, QC], bf16, tag=f"rcb{h}",
                                  name=f"rcb{h}", bufs=2)
                nc.vector.tensor_copy(out=rrb[:], in_=rr[:])
                rrows.append(rrb)

            # heads 0,1: QK row-paired via partition bases 0/64.
            # PV is emitted one jp behind the scores so the exp+mask latency
            # of block jp hides behind the score matmuls of block jp+1
            # (PE executes its queue in order).
            def pv01(item):
                pjs, pts = item
                for h in range(2):
                    for si, j in enumerate(pjs):
                        nc.tensor.matmul(
                            yps01[h][:],
                            lhsT=vones[h][:, j * VS:j * VS + HD + 1],
                            rhs=pts[h][:, si * QC:(si + 1) * QC],
                            start=(j == 0), stop=(j == nkb - 1))

            yps01 = [y_ps.tile([65, QC], f32, tag="y", name="y")
                     for _ in range(2)]
            prev01 = None
            for jp in range(nkb // 2):
                js = (2 * jp, 2 * jp + 1)
                s01 = [s_ps.tile([128, 2 * QC], f32, tag="s", name="s")
                       for _ in range(2)]
                for si, j in enumerate(js):
                    for h in range(2):   # adjacent emission -> concurrent
                        nc.tensor.matmul(
                            s01[h][:, si * QC:(si + 1) * QC],
                            lhsT=kmap[h][:, j * KB:(j + 1) * KB],
                            rhs=qmap[h][:, dsl],
                            start=True, stop=True)
                if prev01 is not None:
                    pv01(prev01)
                if jp == 1 and pending:
                    # finish the previous chunk mid-attention: its PE/DVE work
                    # fills the exp-latency bubbles of the first score blocks
                    # and keeps the post-attention tail short.
                    finish_chunk(pending.pop(0))
                pts = []
                for h in range(2):
                    pt = p_pool.tile([128, 2 * QC], bf16, tag="p", name="p")
                    nc.scalar.activation(pt[:], s01[h][:], Exp)
                    apply_masks(pt, js)
                    pts.append(pt)
                prev01 = (js, pts)
            pv01(prev01)
            for h in range(2):
                drain_y(h, yps01[h])

            # head 2: QK paired across even/odd k-blocks
            # even j: k2@E2[0:64] x q2@Cqk[0:64]; odd j: k2@Cqk[64:128]
            # x q2@E2[64:128]
            def pv2(item):
                pjs, pt = item
                for si, j in enumerate(pjs):
                    nc.tensor.matmul(
                        yps2[:],
                        lhsT=vones[2][:, j * VS:j * VS + HD + 1],
                        rhs=pt[:, si * QC:(si + 1) * QC],
                        start=(j == 0), stop=(j == nkb - 1))

            yps2 = y_ps.tile([65, QC], f32, tag="y", name="y")
            prev2 = None
            for jp in range(nkb // 2):
                js = (2 * jp, 2 * jp + 1)
                sps = s_ps.tile([128, 2 * QC], f32, tag="s", name="s")
                nc.tensor.matmul(sps[:, 0:QC],
                                 lhsT=E2[0:64, js[0] * KB:(js[0] + 1) * KB],
                                 rhs=Cqk[0:64, dsl], start=True, stop=True)
                nc.tensor.matmul(sps[:, QC:2 * QC],
                                 lhsT=Cqk[64:128,
                                          js[1] * KB:(js[1] + 1) * KB],
                                 rhs=E2[64:128, dsl], start=True, stop=True)
                if prev2 is not None:
                    pv2(prev2)
                pt = p_pool.tile([128, 2 * QC], bf16, tag="p", name="p")
                nc.scalar.activation(pt[:], sps[:], Exp)
                apply_masks(pt, js)
                prev2 = (js, pt)
            pv2(prev2)
            drain_y(2, yps2)

            if pending:   # only reached for the nkb==4 first chunk shapes
                finish_chunk(pending.pop(0))
            pending.append((qc, dsl, rrows))

        while pending:
            finish_chunk(pending.pop(0))


# ---------------------------------------------------------------------------
# host-side sharding / unsharding
# ---------------------------------------------------------------------------

def tf32_round(a):
    """Round fp32 array to tf32 (fp32r): RNE to 10 mantissa bits."""
    b = np.ascontiguousarray(a, dtype=np.float32).view(np.uint32).copy()
    b += 0x0FFF + ((b >> 13) & 1)
    b &= np.uint32(0xFFFFE000)
    return b.view(np.float32)


def _core_cols(h0):
    """wqkv column order per core: [q0 q1 q2 k2 k0 k1 v0 v1 v2] (local)."""
    idx = []
    for blk, l in [(0, 0), (0, 1), (0, 2), (1, 2), (1, 0), (1, 1),
                   (2, 0), (2, 1), (2, 2)]:
        g = h0 + l
        idx.append(np.arange(HD) + blk * C + g * HD)
    return np.concatenate(idx)


def make_masks():
    import ml_dtypes
    m = np.zeros((KB, 4 * QC), dtype=np.float32)
    kp = np.arange(KB)[:, None]
    qf = np.arange(QC)[None, :]
    for mi in range(4):
        m[:, mi * QC:(mi + 1) * QC] = (kp <= qf - 128 * mi)
    return m.astype(ml_dtypes.bfloat16)


def shard_inputs(x, w_attn, b_attn):
    import ml_dtypes
    bf = ml_dtypes.bfloat16
    x = np.ascontiguousarray(np.asarray(x, dtype=np.float32))
    w_attn = np.asarray(w_attn, dtype=np.float32)
    b_attn = np.asarray(b_attn, dtype=np.float32)
    masks = make_masks()
    eye = np.vstack([np.eye(HD, dtype=np.float32)] * 2).astype(bf)
    # packed layouts: one DMA per tensor on-device.
    # xT[p, kt, t] = x.T[kt*128+p, t];  wqkv[p, kt*KW+c] = wq[kt*128+p, c]
    xTb = [np.ascontiguousarray(
        x[b].T.reshape(6, 128, T).transpose(1, 0, 2)).astype(bf)
        for b in range(B)]
    in_maps = []
    for c in range(NCORES):
        b = c // 4
        h0 = HPC * (c % 4)
        cols = _core_cols(h0)
        wq = w_attn[:, cols].copy()
        bq = b_attn[cols].copy()
        # fold 1/sqrt(HD)=0.125 into the q columns (exact power of two)
        wq[:, 0:HPC * HD] *= 0.125
        bq[0:HPC * HD] *= 0.125
        wqp = np.ascontiguousarray(
            wq.reshape(6, 128, KW).transpose(1, 0, 2).reshape(128, 6 * KW))
        bqp = np.zeros((128, 5), dtype=np.float32)
        for m in range(5):
            pm = 128 if m < 4 else 64
            bqp[0:pm, m] = bq[m * 128:m * 128 + pm]
        in_maps.append({
            "xT": xTb[b],
            "wqkv": wqp.astype(bf),
            "bqkv": bqp,
            "wp": None,  # filled below
            "masks": masks,
            "eye": eye,
        })
    return in_maps


def fill_wp(in_maps, w_proj):
    import ml_dtypes
    w_proj = np.asarray(w_proj, dtype=np.float32)
    for c in range(NCORES):
        h0 = HPC * (c % 4)
        rows = np.concatenate(
            [np.arange(HD) + (h0 + l) * HD for l in range(HPC)])
        in_maps[c]["wp"] = np.ascontiguousarray(w_proj[rows, :]).astype(
            ml_dtypes.bfloat16)
    return in_maps


def gather_outputs(results, b_proj):
    b_proj = np.asarray(b_proj, dtype=np.float32)
    y = np.zeros((B, T, C), dtype=np.float32)
    for c in range(NCORES):
        b = c // 4
        # zT[p, mc, t] -> z[mc*128+p, t] -> (T, C)
        z = results[c]["zT"].astype(np.float32)
        y[b] += z.transpose(1, 0, 2).reshape(C, T).T
    y += b_proj[None, None, :]
    return y


_NC_CACHE = {}


def get_nc():
    if "nc" not in _NC_CACHE:
        _NC_CACHE["nc"] = build_program()
    return _NC_CACHE["nc"]


def run_spmd(in_maps, trace=False, **kw):
    from concourse.bass_utils import run_bass_kernel_spmd
    nc = get_nc()
    return run_bass_kernel_spmd(nc, in_maps, core_ids=list(range(NCORES)),
                                trace=trace, **kw)


def kernel(x, w_attn, b_attn, w_proj, b_proj):
    in_maps = shard_inputs(x, w_attn, b_attn)
    fill_wp(in_maps, w_proj)
    res = run_spmd(in_maps)
    return gather_outputs(res.results, b_proj)



# revision 41
# speedup vs baseline: 1.0347x; 1.0347x over previous
"""Causal self-attention (B=2, T=4096, C=768, NH=12) on 8 trn2 NeuronCores.

Sharding: data parallel on B (cores 0-3 -> batch 0, cores 4-7 -> batch 1),
tensor parallel on heads (3 heads per core).  Each core computes, for its
batch b and heads [h0, h0+1, h0+2]:
  qkv^T = (x_b @ W_slice)^T           (f32r matmuls, W stationary)
  per head: causal attention entirely in SBUF
      s^T[k,q] = k^T . q^T   (scores transposed; softmax denom via a ones
      column appended to v so one PV matmul yields both y^T and denom)
  y^T normalized via Newton-iteration reciprocal + PE broadcast
  z^T_partial = (y @ Wp_rows)^T       (per-core partial of output proj)
Host sums the 4 partials per batch and adds the biases' contribution.

The whole kernel is one software pipeline over the 8 token chunks:
qkv-projection(chunk) -> v-transposes(chunk) -> attention(q-chunk) ->
normalize -> z-projection(chunk), so PE / ACT / DVE / DMA overlap
throughout instead of phase-by-phase.

All matmuls use dt.float32r (tf32: 1 cyc/row at N>=256, fp32 accumulate).
"""

import sys

for _p in ("/opt/trn_rl_repo",):
    if _p not in sys.path:
        sys.path.insert(0, _p)

import numpy as np
from contextlib import ExitStack

from concourse import bacc, tile, mybir

B, T, C = 2, 4096, 768
NH, HD = 12, 64
NCORES = 8
HPC = 3            # heads per core
QC = 512           # q chunk width (N dim of matmuls)
NQC = T // QC      # 8
KB = 128           # k block (partition dim of transposed scores)
NKB = T // KB      # 32
KW = 3 * HPC * HD  # 576 qkv columns per core
VS = HD + 4        # vones block stride (8B-aligned for bf16)
f32 = mybir.dt.float32
f32r = mybir.dt.float32r
bf16 = mybir.dt.bfloat16
u32 = mybir.dt.uint32
u16 = mybir.dt.uint16

MAGIC = 0x7EF311C3  # fp32 reciprocal seed: r0 = bits(MAGIC - bits(x))


def build_program():
    nc = bacc.Bacc("TRN2", target_bir_lowering=False, debug=False,
                   num_devices=NCORES)
    xT = nc.dram_tensor("xT", [128, 6, T], bf16, kind="ExternalInput").ap()
    wqkv = nc.dram_tensor("wqkv", [128, 6 * KW], bf16,
                          kind="ExternalInput").ap()
    bqkv = nc.dram_tensor("bqkv", [128, 5], f32, kind="ExternalInput").ap()
    wp = nc.dram_tensor("wp", [HPC * HD, C], bf16, kind="ExternalInput").ap()
    masks = nc.dram_tensor("masks", [KB, 4 * QC], bf16,
                           kind="ExternalInput").ap()
    eye = nc.dram_tensor("eye", [2 * HD, HD], bf16, kind="ExternalInput").ap()
    zT = nc.dram_tensor("zT", [128, 6, T], bf16,
                        kind="ExternalOutput").ap()

    with tile.TileContext(nc) as tc:
        _body(tc, xT, wqkv, bqkv, wp, masks, eye, zT)
    nc.compile()
    return nc


def _body(tc, xT, wqkv, bqkv, wp, masks, eye, zT):
    nc = tc.nc
    Exp = mybir.ActivationFunctionType.Exp
    mult = mybir.AluOpType.mult
    sub = mybir.AluOpType.subtract
    add = mybir.AluOpType.add

    with ExitStack() as ctx:
        # ---------------- SBUF pools (all live for the whole pipeline) ----
        qk_pool = ctx.enter_context(tc.tile_pool(name="qk", bufs=4))
        y_pool = ctx.enter_context(tc.tile_pool(name="yt", bufs=2))
        wp_pool = ctx.enter_context(tc.tile_pool(name="wp", bufs=2))
        wq_pool = ctx.enter_context(tc.tile_pool(name="wq", bufs=1))
        mask_pool = ctx.enter_context(tc.tile_pool(name="mask", bufs=1))
        const_pool = ctx.enter_context(tc.tile_pool(name="const", bufs=1))
        bias_pool = ctx.enter_context(tc.tile_pool(name="bias", bufs=1))
        vones_pool = ctx.enter_context(tc.tile_pool(name="vones", bufs=1))
        xk_pool = ctx.enter_context(tc.tile_pool(name="xk", bufs=2))
        vt_pool = ctx.enter_context(tc.tile_pool(name="vt", bufs=2))
        p_pool = ctx.enter_context(tc.tile_pool(name="p", bufs=6))
        z_pool = ctx.enter_context(tc.tile_pool(name="z", bufs=2))
        d_pool = ctx.enter_context(tc.tile_pool(name="d", bufs=1))
        # PSUM pools: 4 + 2 + 2 = 8 banks
        s_ps = ctx.enter_context(tc.tile_pool(name="sps", bufs=2, space="PSUM"))
        y_ps = ctx.enter_context(tc.tile_pool(name="yps", bufs=2, space="PSUM"))
        mm_ps = ctx.enter_context(tc.tile_pool(name="mmps", bufs=2,
                                               space="PSUM"))

        # persistent tiles
        # q^T/k^T layout: A=[q_l0;q_l1] B=[k_l0;k_l1] Cqk=[q_l2;k_l2]
        # E2 = copy of k_l2 at base partition 0 (q2 and k2 must share a base)
        A = qk_pool.tile([128, T], bf16, tag="qk", name="qk")
        Bt = qk_pool.tile([128, T], bf16, tag="qk", name="qk")
        Cqk = qk_pool.tile([128, T], bf16, tag="qk", name="qk")
        E2 = qk_pool.tile([128, T], bf16, tag="qk", name="qk")
        F = y_pool.tile([128, T], bf16, tag="yt", name="yt")   # y^T l0,l1
        G = y_pool.tile([64, T], bf16, tag="yt", name="yt")    # y^T l2
        wp0 = wp_pool.tile([128, C], bf16, tag="wp", name="wp")
        wp1 = wp_pool.tile([64, C], bf16, tag="wp", name="wp")
        mask_t = mask_pool.tile([KB, 4 * QC], bf16, tag="mask", name="mask")
        eye_t = const_pool.tile([2 * HD, HD], bf16, tag="eye", name="eye")
        ones_pe = const_pool.tile([1, 64], bf16, tag="ones", name="ones")
        vones = [vones_pool.tile([128, NKB * VS], bf16, tag=f"vones{i}",
                                 name=f"vones{i}") for i in range(HPC)]

        wq_all = wq_pool.tile([128, 6 * KW], bf16, tag="wq", name="wq")
        nc.sync.dma_start(out=wq_all[:], in_=wqkv[:])
        wq_t = [wq_all[:, kt * KW:(kt + 1) * KW] for kt in range(6)]
        bias_all = bias_pool.tile([128, 5], f32, tag="bias", name="bias")
        nc.sync.dma_start(out=bias_all[:], in_=bqkv[:])
        bias_t = [bias_all[0:(128 if m < 4 else 64), m:m + 1]
                  for m in range(5)]
        nc.sync.dma_start(out=eye_t[:], in_=eye[:])
        nc.gpsimd.memset(ones_pe[:].bitcast(u16), 0x3F80)  # +1.0bf
        for h in range(HPC):
            nc.gpsimd.memset(vones[h][:].bitcast(u16), 0x3F80)  # 1.0bf

        qmap = [A[0:64, :], A[64:128, :], Cqk[0:64, :]]
        kmap = [Bt[0:64, :], Bt[64:128, :], E2[0:64, :]]

        def drain_y(yps, dsl, h, rrows):
            ydst = (F[0:64, dsl], F[64:128, dsl], G[0:64, dsl])[h]
            nc.vector.tensor_copy(out=ydst, in_=yps[0:64, :])
            rr = d_pool.tile([1, QC], f32, tag=f"rcp{h}",
                             name=f"rcp{h}", bufs=2)
            nc.vector.reciprocal(out=rr[:], in_=yps[64:65, :])
            rrb = d_pool.tile([1, QC], bf16, tag=f"rcb{h}",
                              name=f"rcb{h}", bufs=2)
            nc.vector.tensor_copy(out=rrb[:], in_=rr[:])
            rrows.append(rrb)

        def finish_chunk(item):
            qc, dsl, rrows = item
            for h in range(HPC):
                rb = mm_ps.tile([128, QC], f32, tag="mm", name="mm")
                nc.tensor.matmul(rb[0:64, :], lhsT=ones_pe[:],
                                 rhs=rrows[h][:], start=True, stop=True)
                if h == 0:
                    ydst = F[0:64, dsl]
                elif h == 1:
                    ydst = F[64:128, dsl]
                else:
                    ydst = G[0:64, dsl]
                nc.vector.tensor_tensor(out=ydst, in0=ydst, in1=rb[0:64, :],
                                        op=mult)
            zbig = z_pool.tile([128, 6 * QC], bf16, tag="z", name="z")
            Copy = mybir.ActivationFunctionType.Copy
            for mc in range(6):
                zp = mm_ps.tile([128, QC], f32, tag="mm", name="mm")
                nc.tensor.matmul(zp[:],
                                 lhsT=wp0[:, mc * 128:(mc + 1) * 128],
                                 rhs=F[:, dsl], start=True, stop=False)
                nc.tensor.matmul(zp[:],
                                 lhsT=wp1[:, mc * 128:(mc + 1) * 128],
                                 rhs=G[:, dsl], start=False, stop=True)
                dst = zbig[:, mc * QC:(mc + 1) * QC]
                if mc % 2 == 0:   # split PSUM drains across DVE and ACT
                    nc.vector.tensor_copy(out=dst, in_=zp[:])
                else:
                    nc.scalar.activation(dst, zp[:], Copy)
            nc.sync.dma_start(out=zT[:, :, dsl], in_=zbig[:])

        # ================= the pipeline over token chunks =================
        pending = []
        ydefer = []
        for tci in range(NQC):
            csl = slice(tci * QC, (tci + 1) * QC)

            # ---- qkv^T projection for this chunk (one packed DMA) ----
            xk_all = xk_pool.tile([128, 6 * QC], bf16, tag="xk", name="xk")
            nc.sync.dma_start(out=xk_all[:], in_=xT[:, :, csl])
            if tci == 0:
                # off the lead-in critical path: masks are first consumed by
                # attention(0) (after qkv proj), wp only by finish_chunk(0)
                # during iteration 1.
                nc.sync.dma_start(out=mask_t[:], in_=masks[:])
                nc.sync.dma_start(out=wp0[:], in_=wp[0:128, :])
                nc.sync.dma_start(out=wp1[:], in_=wp[128:192, :])
            vchunk = vt_pool.tile([128, QC], bf16, tag="vt", name="vt")
            vchunk2 = vt_pool.tile([64, QC], bf16, tag="vt2", name="vt2")
            mdst = [(A, 0, 128), (Cqk, 0, 128), (Bt, 0, 128),
                    (vchunk, 0, 128), (vchunk2, 0, 64)]
            for m in range(5):
                dst, roff, pm = mdst[m]
                ps = mm_ps.tile([128, QC], f32, tag="mm", name="mm")
                for kt in range(6):
                    nc.tensor.matmul(
                        ps[0:pm, :],
                        lhsT=wq_t[kt][:, m * 128:m * 128 + pm],
                        rhs=xk_all[:, kt * QC:(kt + 1) * QC],
                        start=(kt == 0), stop=(kt == 5))
                out_sl = dst[roff:roff + pm, csl] if m < 3 else dst[0:pm, :]
                nc.vector.tensor_scalar(out_sl, ps[0:pm, :], bias_t[m][:],
                                        None, add)
                if m == 1:   # k_l2 at base 0 and q_l2 at base 64 (pairing)
                    nc.vector.tensor_scalar(
                        E2[0:64, csl], ps[64:128, :],
                        bias_t[1][64:128, :], None, add)
                    nc.vector.tensor_scalar(
                        E2[64:128, csl], ps[0:64, :],
                        bias_t[1][0:64, :], None, add)

            # ---- v transposes for this chunk's 4 k-blocks ----
            vmap = [vchunk[0:64, :], vchunk[64:128, :], vchunk2[0:64, :]]
            for jj in range(4):
                j = 4 * tci + jj
                tps = []
                for h in range(2):   # h0/h1 adjacent -> row-concurrent
                    base = vmap[h].base_partition()
                    tp = mm_ps.tile([128, HD], bf16, tag="mm", name="mm")
                    nc.tensor.transpose(
                        tp[:], vmap[h][:, jj * KB:(jj + 1) * KB],
                        eye_t[base:base + HD, :])
                    tps.append(tp)
                for h in range(2):
                    nc.vector.tensor_copy(
                        out=vones[h][:, j * VS:j * VS + HD], in_=tps[h][:])
                tp2 = mm_ps.tile([128, HD], bf16, tag="mm", name="mm")
                nc.tensor.transpose(tp2[:], vmap[2][:, jj * KB:(jj + 1) * KB],
                                    eye_t[0:HD, :])
                nc.vector.tensor_copy(
                    out=vones[2][:, j * VS:j * VS + HD], in_=tp2[:])

            # previous chunk's normalize drains go here, AFTER this
            # chunk's projection drains are queued on DVE (in-order queue).
            if ydefer:
                pdsl, pyps, prr = ydefer.pop(0)
                for h in range(HPC):
                    drain_y(pyps[h], pdsl, h, prr)

            # ---- attention for q chunk qc == tci ----
            qc = tci
            dsl = csl
            nkb = 4 * qc + 4
            rrows = []

            def expblk(pt, sps, js):
                ms = [max(j - 4 * qc, 0) for j in js]
                if ms[0] == 0 and ms[1] == 0:
                    nc.scalar.activation(pt[:], sps[:], Exp)
                else:
                    for si, m in enumerate(ms):
                        sl = slice(si * QC + 128 * m, (si + 1) * QC)
                        nc.scalar.activation(pt[:, sl], sps[:, sl], Exp)

            def apply_masks(pt, js):
                for si, j in enumerate(js):
                    m = j - 4 * qc
                    if m >= 0:   # diagonal block: apply causal mask over
                        # the computed (truncated) region only
                        nc.vector.tensor_tensor(
                            out=pt[:, si * QC + 128 * m:(si + 1) * QC],
                            in0=pt[:, si * QC + 128 * m:(si + 1) * QC],
                            in1=mask_t[:, m * QC + 128 * m:(m + 1) * QC],
                            op=mult)

            # heads 0,1: QK row-paired via partition bases 0/64.
            # PV is emitted one jp behind the scores so the exp+mask latency
            # of block jp hides behind the score matmuls of block jp+1
            # (PE executes its queue in order).
            def pv01(item):
                pjs, pts = item
                for h in range(2):
                    for si, j in enumerate(pjs):
                        m = max(j - 4 * qc, 0)
                        nc.tensor.matmul(
                            yps01[h][:, 128 * m:],
                            lhsT=vones[h][:, j * VS:j * VS + HD + 1],
                            rhs=pts[h][:, si * QC + 128 * m:(si + 1) * QC],
                            start=(j == 0), stop=(j == nkb - 1))

            yps01 = [y_ps.tile([65, QC], f32, tag="y", name="y")
                     for _ in range(2)]
            prev01 = None
            for jp in range(nkb // 2):
                js = (2 * jp, 2 * jp + 1)
                s01 = [s_ps.tile([128, 2 * QC], f32, tag="s", name="s")
                       for _ in range(2)]
                for si, j in enumerate(js):
                    m = max(j - 4 * qc, 0)
                    qsl = slice(qc * QC + 128 * m, (qc + 1) * QC)
                    for h in range(2):   # adjacent emission -> concurrent
                        nc.tensor.matmul(
                            s01[h][:, si * QC + 128 * m:(si + 1) * QC],
                            lhsT=kmap[h][:, j * KB:(j + 1) * KB],
                            rhs=qmap[h][:, qsl],
                            start=True, stop=True)
                if prev01 is not None:
                    pv01(prev01)
                if jp == 1 and pending:
                    # finish the previous chunk mid-attention: its PE/DVE work
                    # fills the exp-latency bubbles of the first score blocks
                    # and keeps the post-attention tail short.
                    finish_chunk(pending.pop(0))
                pts = []
                for h in range(2):
                    pt = p_pool.tile([128, 2 * QC], bf16, tag="p", name="p")
                    expblk(pt, s01[h], js)
                    apply_masks(pt, js)
                    pts.append(pt)
                prev01 = (js, pts)
            pv01(prev01)

            # head 2: QK paired across even/odd k-blocks
            # even j: k2@E2[0:64] x q2@Cqk[0:64]; odd j: k2@Cqk[64:128]
            # x q2@E2[64:128]
            def pv2(item):
                pjs, pt = item
                for si, j in enumerate(pjs):
                    m = max(j - 4 * qc, 0)
                    nc.tensor.matmul(
                        yps2[:, 128 * m:],
                        lhsT=vones[2][:, j * VS:j * VS + HD + 1],
                        rhs=pt[:, si * QC + 128 * m:(si + 1) * QC],
                        start=(j == 0), stop=(j == nkb - 1))

            yps2 = y_ps.tile([65, QC], f32, tag="y", name="y")
            prev2 = None
            for jp in range(nkb // 2):
                js = (2 * jp, 2 * jp + 1)
                sps = s_ps.tile([128, 2 * QC], f32, tag="s", name="s")
                m0 = max(js[0] - 4 * qc, 0)
                m1 = max(js[1] - 4 * qc, 0)
                nc.tensor.matmul(sps[:, 128 * m0:QC],
                                 lhsT=E2[0:64, js[0] * KB:(js[0] + 1) * KB],
                                 rhs=Cqk[0:64, qc * QC + 128 * m0:
                                         (qc + 1) * QC],
                                 start=True, stop=True)
                nc.tensor.matmul(sps[:, QC + 128 * m1:2 * QC],
                                 lhsT=Cqk[64:128,
                                          js[1] * KB:(js[1] + 1) * KB],
                                 rhs=E2[64:128, qc * QC + 128 * m1:
                                        (qc + 1) * QC],
                                 start=True, stop=True)
                if prev2 is not None:
                    pv2(prev2)
                pt = p_pool.tile([128, 2 * QC], bf16, tag="p", name="p")
                expblk(pt, sps, js)
                apply_masks(pt, js)
                prev2 = (js, pt)
            pv2(prev2)

            if pending:   # only reached for the nkb==4 first chunk shapes
                finish_chunk(pending.pop(0))
            ydefer.append((dsl, [yps01[0], yps01[1], yps2], rrows))
            pending.append((qc, dsl, rrows))

        for pdsl, pyps, prr in ydefer:
            for h in range(HPC):
                drain_y(pyps[h], pdsl, h, prr)
        while pending:
            finish_chunk(pending.pop(0))


# ---------------------------------------------------------------------------
# host-side sharding / unsharding
# ---------------------------------------------------------------------------

def tf32_round(a):
    """Round fp32 array to tf32 (fp32r): RNE to 10 mantissa bits."""
    b = np.ascontiguousarray(a, dtype=np.float32).view(np.uint32).copy()
    b += 0x0FFF + ((b >> 13) & 1)
    b &= np.uint32(0xFFFFE000)
    return b.view(np.float32)


def _core_cols(h0):
    """wqkv column order per core: [q0 q1 q2 k2 k0 k1 v0 v1 v2] (local)."""
    idx = []
    for blk, l in [(0, 0), (0, 1), (0, 2), (1, 2), (1, 0), (1, 1),
                   (2, 0), (2, 1), (2, 2)]:
        g = h0 + l
        idx.append(np.arange(HD) + blk * C + g * HD)
    return np.concatenate(idx)


def make_masks():
    import ml_dtypes
    m = np.zeros((KB, 4 * QC), dtype=np.float32)
    kp = np.arange(KB)[:, None]
    qf = np.arange(QC)[None, :]
    for mi in range(4):
        m[:, mi * QC:(mi + 1) * QC] = (kp <= qf - 128 * mi)
    return m.astype(ml_dtypes.bfloat16)


def shard_inputs(x, w_attn, b_attn):
    import ml_dtypes
    bf = ml_dtypes.bfloat16
    x = np.ascontiguousarray(np.asarray(x, dtype=np.float32))
    w_attn = np.asarray(w_attn, dtype=np.float32)
    b_attn = np.asarray(b_attn, dtype=np.float32)
    masks = make_masks()
    eye = np.vstack([np.eye(HD, dtype=np.float32)] * 2).astype(bf)
    # packed layouts: one DMA per tensor on-device.
    # xT[p, kt, t] = x.T[kt*128+p, t];  wqkv[p, kt*KW+c] = wq[kt*128+p, c]
    xTb = [np.ascontiguousarray(
        x[b].T.reshape(6, 128, T).transpose(1, 0, 2)).astype(bf)
        for b in range(B)]
    in_maps = []
    for c in range(NCORES):
        b = c // 4
        h0 = HPC * (c % 4)
        cols = _core_cols(h0)
        wq = w_attn[:, cols].copy()
        bq = b_attn[cols].copy()
        # fold 1/sqrt(HD)=0.125 into the q columns (exact power of two)
        wq[:, 0:HPC * HD] *= 0.125
        bq[0:HPC * HD] *= 0.125
        wqp = np.ascontiguousarray(
            wq.reshape(6, 128, KW).transpose(1, 0, 2).reshape(128, 6 * KW))
        bqp = np.zeros((128, 5), dtype=np.float32)
        for m in range(5):
            pm = 128 if m < 4 else 64
            bqp[0:pm, m] = bq[m * 128:m * 128 + pm]
        in_maps.append({
            "xT": xTb[b],
            "wqkv": wqp.astype(bf),
            "bqkv": bqp,
            "wp": None,  # filled below
            "masks": masks,
            "eye": eye,
        })
    return in_maps


def fill_wp(in_maps, w_proj):
    import ml_dtypes
    w_proj = np.asarray(w_proj, dtype=np.float32)
    for c in range(NCORES):
        h0 = HPC * (c % 4)
        rows = np.concatenate(
            [np.arange(HD) + (h0 + l) * HD for l in range(HPC)])
        in_maps[c]["wp"] = np.ascontiguousarray(w_proj[rows, :]).astype(
            ml_dtypes.bfloat16)
    return in_maps


def gather_outputs(results, b_proj):
    b_proj = np.asarray(b_proj, dtype=np.float32)
    y = np.zeros((B, T, C), dtype=np.float32)
    for c in range(NCORES):
        b = c // 4
        # zT[p, mc, t] -> z[mc*128+p, t] -> (T, C)
        z = results[c]["zT"].astype(np.float32)
        y[b] += z.transpose(1, 0, 2).reshape(C, T).T
    y += b_proj[None, None, :]
    return y


_NC_CACHE = {}


def get_nc():
    if "nc" not in _NC_CACHE:
        _NC_CACHE["nc"] = build_program()
    return _NC_CACHE["nc"]


def run_spmd(in_maps, trace=False, **kw):
    from concourse.bass_utils import run_bass_kernel_spmd
    nc = get_nc()
    return run_bass_kernel_spmd(nc, in_maps, core_ids=list(range(NCORES)),
                                trace=trace, **kw)


def kernel(x, w_attn, b_attn, w_proj, b_proj):
    in_maps = shard_inputs(x, w_attn, b_attn)
    fill_wp(in_maps, w_proj)
    res = run_spmd(in_maps)
    return gather_outputs(res.results, b_proj)

